# revision 1
# baseline (speedup 1.0000x reference)
"""Trainium2 Bass kernel for AbsoluteSinusoidal2DPE logits.

Math (flattened, N = H*W = 1024, D = 512):
    logits[b] = q[b] @ e^T + e @ (k[b] + e)^T          # [N, N] per batch

Sharding: batch dim (16) data-parallel over 8 cores, 2 batches/core; the
[N, D] embed table is replicated. Host-side prep transposes operands to
[D, N] (contraction dim on partitions) and rounds matmul operands to the
PE's fp32r format (fp32 with 11 explicit mantissa bits, single-pass
matmul at 1 column/cycle vs 4 for full fp32).

Per core: 256 matmuls (K=128, M=128, N=512). Measured steady state is at
the PE streaming floor (~34 us/batch = 128 MMs x 512 cols at the chip's
sustained ~2.0 GHz). Startup is a quadrant schedule (PSUM banks split 4/4
between m-halves, pass order A1 B1 C1 A2 D1 B2 C2 D2) whose operand
consumption order exactly matches the DMA arrival order, with a PE
pre-warm so the HAM clock gate is at full rate when real matmuls start.
Batch 0's output is staged fully in SBUF and flushed as one 4MB DMA so
batch 1's input loads own the DMA bandwidth (no mid-kernel contention);
batch 1 trickles outputs group-serially and splits the last stores
across both HWDGE rings. Relative error vs the fp32 reference ~1e-4
(absmax/scale ~4.5e-5).
"""

import numpy as np

B, H, W, D = 16, 32, 32, 512
N = H * W            # 1024
NCORES = 8
BPC = B // NCORES    # batches per core
P = 128              # partitions
KO = D // P          # 4 contraction chunks
NT = N // P          # 8 output row tiles
MH = N // 512        # 2 output column halves (PSUM bank = 512 fp32)

_PROG = None  # cached (nc) bass program, reused across kernel() calls


def _round_fp32r(x: np.ndarray) -> np.ndarray:
    """Round fp32 -> fp32r (RNE to 11 explicit mantissa bits, low 12 bits 0).

    Matches TRN2 hardware rounding (verified against DVE f32->f32r copy).
    """
    xi = x.view(np.uint32).astype(np.uint64)
    add = ((xi >> 12) & 1) + 0x7FF
    xi = (xi + add) & 0xFFFFF000
    return np.ascontiguousarray(xi.astype(np.uint32).view(np.float32))


def _build_program(n_batches: int = BPC, loop_reps: int = 0,
                   startup: str = "quadrant", prewarm: bool = True):
    """n_batches > BPC repeats the batch loop (cycling the same DRAM data);
    loop_reps > 0 wraps the whole body in a For_i hardware loop (timing
    instrument); startup="simple" disables the quadrant/pre-warm startup
    (A/B baseline). The real kernel uses the defaults."""
    import contextlib
    import concourse.mybir as mybir
    import concourse.tile as tile
    from concourse import bacc

    F32 = mybir.dt.float32
    F32R = mybir.dt.float32r

    nc = bacc.Bacc()
    qt_d = nc.dram_tensor("qt", [BPC, D, N], F32R, kind="ExternalInput")
    kt_d = nc.dram_tensor("kt", [BPC, D, N], F32, kind="ExternalInput")
    et_d = nc.dram_tensor("et", [D, N], F32R, kind="ExternalInput")
    out_d = nc.dram_tensor("out", [BPC, N, N], F32, kind="ExternalOutput")

    with tile.TileContext(nc) as tc:
        with (
            tc.tile_pool(name="etp", bufs=1) as etp,
            tc.tile_pool(name="inp", bufs=2) as inp,
            tc.tile_pool(name="outp", bufs=8) as outp,
            tc.tile_pool(name="stg", bufs=1) as stg,
            tc.tile_pool(name="ps", bufs=1, space="PSUM") as psp,
        ):
          loop_cm = tc.For_i(0, loop_reps, 1) if loop_reps else contextlib.nullcontext()
          with loop_cm:
            # embed^T resident: [128, KO, N]
            et = etp.tile([P, KO, N], F32R, name="et")
            et_src = et_d.rearrange("(ko p) m -> p ko m", p=P)

            if startup == "quadrant" and prewarm:
                # PE pre-warm: dummy matmuls on a zeroed scratch tile while
                # the first input DMAs are in flight, so the HAM clock gate
                # reaches full rate before real matmuls start
                warm = etp.tile([P, 128], F32R, name="warm")
                nc.vector.memset(warm[:].bitcast(F32), 0.0)
                warm_ps = psp.tile([P, 512], F32, tag="ps7", name="warm_ps")
                for _ in range(16):
                    nc.tensor.matmul(warm_ps[:, 0:128], warm[:], warm[:],
                                     start=True, stop=True)

            for b in range(n_batches):
                bi = b % BPC
                qt = inp.tile([P, KO, N], F32R, tag="qt")
                kt = inp.tile([P, KO, N], F32, tag="kt")
                kpe = inp.tile([P, KO, N], F32R, tag="kpe")
                qt_src = qt_d[bi].rearrange("(ko p) m -> p ko m", p=P)
                kt_src = kt_d[bi].rearrange("(ko p) m -> p ko m", p=P)
                if b == 0 and startup == "quadrant":
                    # DMA arrival order matched to the quadrant schedule's
                    # consumption order (A1: t1/mh0/nt0-3 -> B1: t1/mh1 ->
                    # C1: t2/mh0 -> D1: t2/mh1 -> half nt4-7). kpe adds are
                    # split per m-half so term2-mh0 only waits on kt[0:512].
                    for ko in range(KO):
                        nc.sync.dma_start(qt[:, ko, 0:512], qt_src[:, ko, 0:512])
                        nc.sync.dma_start(et[:, ko, 0:512], et_src[:, ko, 0:512])
                    for ko in range(KO):
                        nc.sync.dma_start(et[:, ko, 512:N], et_src[:, ko, 512:N])
                    for ko in range(KO):
                        nc.sync.dma_start(kt[:, ko, 0:512], kt_src[:, ko, 0:512])
                        nc.vector.tensor_add(
                            kpe[:, ko, 0:512], kt[:, ko, 0:512],
                            et[:, ko, 0:512].bitcast(F32))
                    for ko in range(KO):
                        nc.sync.dma_start(qt[:, ko, 512:N], qt_src[:, ko, 512:N])
                    for ko in range(KO):
                        nc.sync.dma_start(kt[:, ko, 512:N], kt_src[:, ko, 512:N])
                        nc.vector.tensor_add(
                            kpe[:, ko, 512:N], kt[:, ko, 512:N],
                            et[:, ko, 512:N].bitcast(F32))
                else:
                    if b == 0:  # simple-startup baseline: load et here
                        for ko in range(KO):
                            nc.sync.dma_start(et[:, ko], et_src[:, ko])
                    # qt ahead of kt: the batch's term1 (first 32 matmuls)
                    # needs all qt chunks; kpe (term2) is consumed ~7us later
                    for ko in range(KO):
                        nc.sync.dma_start(qt[:, ko], qt_src[:, ko])
                    for ko in range(KO):
                        nc.sync.dma_start(kt[:, ko], kt_src[:, ko])
                        # (k + e) rounded to fp32r via DVE output dtype
                        nc.vector.tensor_add(
                            kpe[:, ko], kt[:, ko], et[:, ko].bitcast(F32)
                        )

                out_rows = out_d[bi].rearrange("(nt p) m -> nt p m", p=P)

                def mm_t1(ps, nt, ko, ms, start):
                    nc.tensor.matmul(
                        ps[:], qt[:, ko, nt * P:(nt + 1) * P], et[:, ko, ms],
                        start=start, stop=False)

                def mm_t2(ps, nt, ko, ms, stop):
                    nc.tensor.matmul(
                        ps[:], et[:, ko, nt * P:(nt + 1) * P], kpe[:, ko, ms],
                        start=False, stop=stop)

                if b == 0 and startup == "quadrant":
                    # startup quadrant schedule, sequenced so each pass's
                    # operands arrive exactly in DMA order:
                    #   A1(t1,mh0,lo) B1(t1,mh1,lo) C1(t2,mh0,lo+close)
                    #   A2(t1,mh0,hi) D1(t2,mh1,lo+close) B2(t1,mh1,hi)
                    #   C2(t2,mh0,hi+close) D2(t2,mh1,hi+close)
                    # mh0 quadrants use banks ps0-3, mh1 quadrants ps4-7
                    lo, hi = list(range(4)), list(range(4, 8))
                    ps_q = {}

                    def open_t1(mh, nts):
                        ms = slice(mh * 512, (mh + 1) * 512)
                        for j, nt in enumerate(nts):
                            tag = f"ps{mh * 4 + j}"
                            ps_q[(mh, nt)] = psp.tile(
                                [P, 512], F32, tag=tag, name=tag)
                        for ko in range(KO):
                            for nt in nts:
                                mm_t1(ps_q[(mh, nt)], nt, ko, ms, ko == 0)

                    # batch-0 output is staged fully in SBUF and flushed as
                    # one 4MB DMA whose dependency (the last close copy)
                    # fires only after batch-1's input loads have the DMA
                    # bandwidth to themselves -- avoids mid-kernel contention
                    ob0 = stg.tile([P, NT, N], F32, name="ob0")

                    def close_t2(mh, nts):
                        ms = slice(mh * 512, (mh + 1) * 512)
                        for ko in range(KO):
                            for nt in nts:
                                mm_t2(ps_q[(mh, nt)], nt, ko, ms, ko == KO - 1)
                        for nt in nts:
                            nc.vector.tensor_copy(ob0[:, nt, ms], ps_q[(mh, nt)][:])

                    open_t1(0, lo)    # A1: qt-h0 + et-m0
                    open_t1(1, lo)    # B1: + et-m1
                    close_t2(0, lo)   # C1: + kpe-m0 (kt-h0)
                    open_t1(0, hi)    # A2: + qt-h1
                    close_t2(1, lo)   # D1: + kpe-m1 (kt-h1)
                    open_t1(1, hi)    # B2
                    close_t2(0, hi)   # C2
                    close_t2(1, hi)   # D2
                    nc.scalar.dma_start(
                        out_d[bi].rearrange("(nt p) m -> p nt m", p=P), ob0[:])
                else:
                    # steady/tail: group-serial so outputs trickle out
                    for mh in range(MH):
                        ms = slice(mh * 512, (mh + 1) * 512)
                        last_pass = (b == n_batches - 1) and (mh == MH - 1)
                        for nt in range(NT):
                            ps = psp.tile([P, 512], F32, tag=f"ps{nt}",
                                          name=f"ps{nt}")
                            for ko in range(KO):
                                mm_t1(ps, nt, ko, ms, ko == 0)
                            for ko in range(KO):
                                mm_t2(ps, nt, ko, ms, ko == KO - 1)
                            ob = outp.tile([P, 512], F32, tag="ob")
                            if last_pass and nt >= NT - 2:
                                # tail: split the final stores across both
                                # HWDGE rings so the last write's HBM receipt
                                # overlaps the other half's stream
                                nc.vector.tensor_copy(ob[:, 0:256], ps[:, 0:256])
                                nc.vector.tensor_copy(ob[:, 256:512], ps[:, 256:512])
                                nc.scalar.dma_start(
                                    out_rows[nt][:, mh * 512:mh * 512 + 256],
                                    ob[:, 0:256])
                                nc.sync.dma_start(
                                    out_rows[nt][:, mh * 512 + 256:(mh + 1) * 512],
                                    ob[:, 256:512])
                            else:
                                nc.vector.tensor_copy(ob[:], ps[:])
                                nc.scalar.dma_start(out_rows[nt][:, ms], ob[:])

    nc.compile()
    return nc


def kernel(q: np.ndarray, k: np.ndarray, embed: np.ndarray) -> np.ndarray:
    global _PROG
    from concourse import bass_utils

    q = np.asarray(q)
    k = np.asarray(k)
    embed = np.asarray(embed)
    assert q.shape == (B, H, W, D) and k.shape == (B, H, W, D)
    assert embed.shape == (H, W, D)

    qf = q.reshape(B, N, D).astype(np.float32, copy=False)
    kf = k.reshape(B, N, D).astype(np.float32, copy=False)
    ef = embed.reshape(N, D).astype(np.float32, copy=False)

    # [B, D, N] contiguous transposes; q and e pre-rounded to fp32r
    qt = _round_fp32r(np.ascontiguousarray(qf.transpose(0, 2, 1)))
    kt = np.ascontiguousarray(kf.transpose(0, 2, 1))
    et = _round_fp32r(np.ascontiguousarray(ef.T))

    if _PROG is None:
        _PROG = _build_program()
    nc = _PROG

    in_maps = []
    for c in range(NCORES):
        sl = slice(c * BPC, (c + 1) * BPC)
        in_maps.append({"qt": qt[sl], "kt": kt[sl], "et": et})

    res = bass_utils.run_bass_kernel_spmd(nc, in_maps, core_ids=list(range(NCORES)))
    outs = [r["out"] for r in res.results]  # each [BPC, N, N]
    full = np.concatenate(outs, axis=0)     # [B, N, N]
    return np.ascontiguousarray(full.reshape(B, H, W, H, W))

